# revision 4
# baseline (speedup 1.0000x reference)
"""BeliefPropagationVC kernel for 8 Trainium2 NeuronCores.

Computes out = 0.5 * ((llr_weight * llr) @ llr_expander.T + input @ (mask * input_weight).T)

Sharding: row-shard the [E, E] mask/input_weight (and [E, NV] llr_expander)
over output edges across the 8 cores; every core keeps the full [B, E] input.
No collectives needed — each core produces out[:, c*EC:(c+1)*EC].

Per-core device kernel (Tile framework):
  - stream k-tiles of mask^T and input_weight^T ([128, KSUB, EC] each),
    multiply elementwise on DVE into a float32r tile, feed that as the
    moving operand of float32r matmuls accumulating into PSUM ([B, 512]),
  - same for llr_expander^T (DVE cast-copy to float32r) against
    (llr_weight*llr)^T,
  - scale by 0.5 on ScalarE and DMA out.

Host side pre-transposes the big matrices (layout prep only; all FLOPs stay
on device) so the contraction dim lands on SBUF partitions.
"""

import numpy as np

B = 32        # batch
E = 8192      # edges (N_VAR * DEG)
NV = 2048     # variable nodes
NCORES = 8
EC = E // NCORES   # 1024 output edges per core
P = 128
KSUB = 2           # k-subtiles (of 128) loaded per DMA -> 1 MiB per transfer
KT = E // (P * KSUB)    # 32 outer k-tiles for the edge-edge matmul
KTL = NV // (P * KSUB)  # 8 outer k-tiles for the llr matmul
NFREE = 512        # matmul moving free dim (one PSUM bank of fp32)
EBLK = EC // NFREE # 2 psum banks

_NC_CACHE = None


def _build_nc():
    from contextlib import ExitStack

    import concourse.bacc as bacc
    import concourse.tile as tile
    from concourse import mybir

    nc = bacc.Bacc("TRN2", target_bir_lowering=False, debug=False)
    f32 = mybir.dt.float32
    f32r = mybir.dt.float32r

    inT = nc.dram_tensor("inT", [E, B], f32, kind="ExternalInput").ap()
    lT = nc.dram_tensor("lT", [NV, B], f32, kind="ExternalInput").ap()
    mT = nc.dram_tensor("mT", [E, EC], f32, kind="ExternalInput").ap()
    wT = nc.dram_tensor("wT", [E, EC], f32, kind="ExternalInput").ap()
    eT = nc.dram_tensor("eT", [NV, EC], f32, kind="ExternalInput").ap()
    out = nc.dram_tensor("out", [B, EC], f32, kind="ExternalOutput").ap()

    # [E, EC] viewed as [KT(outer), KSUB, P(partition), EC]
    mT3 = mT.rearrange("(ko s p) e -> ko s p e", p=P, s=KSUB)
    wT3 = wT.rearrange("(ko s p) e -> ko s p e", p=P, s=KSUB)
    eT3 = eT.rearrange("(ko s p) e -> ko s p e", p=P, s=KSUB)

    with tile.TileContext(nc) as tc, ExitStack() as ctx:
        const = ctx.enter_context(tc.tile_pool(name="const", bufs=1))
        mpool = ctx.enter_context(tc.tile_pool(name="mpool", bufs=3))
        wpool = ctx.enter_context(tc.tile_pool(name="wpool", bufs=3))
        ppool = ctx.enter_context(tc.tile_pool(name="ppool", bufs=3))
        epool = ctx.enter_context(tc.tile_pool(name="epool", bufs=3))
        erpool = ctx.enter_context(tc.tile_pool(name="erpool", bufs=3))
        opool = ctx.enter_context(tc.tile_pool(name="opool", bufs=2))
        psum = ctx.enter_context(tc.tile_pool(name="psum", bufs=1, space="PSUM"))

        # stationary operands, resident for the whole kernel (cast to f32r)
        inT_sb = const.tile([P, E // P, B], f32)
        nc.sync.dma_start(inT_sb[:], inT.rearrange("(ko p) b -> p ko b", p=P))
        inT_r = const.tile([P, E // P, B], f32r)
        nc.vector.tensor_copy(inT_r[:], inT_sb[:])
        lT_sb = const.tile([P, NV // P, B], f32)
        nc.sync.dma_start(lT_sb[:], lT.rearrange("(ko p) b -> p ko b", p=P))
        lT_r = const.tile([P, NV // P, B], f32r)
        nc.vector.tensor_copy(lT_r[:], lT_sb[:])

        acc = [
            psum.tile([B, NFREE], f32, name=f"acc{eb}") for eb in range(EBLK)
        ]

        for ko in range(KT):
            mt = mpool.tile([P, KSUB, EC], f32)
            nc.sync.dma_start(mt[:], mT3[ko].rearrange("s p e -> p s e"))
            wt = wpool.tile([P, KSUB, EC], f32)
            nc.sync.dma_start(wt[:], wT3[ko].rearrange("s p e -> p s e"))
            pt = ppool.tile([P, KSUB, EC], f32r)
            nc.vector.tensor_mul(pt[:], mt[:], wt[:])
            for s in range(KSUB):
                k = ko * KSUB + s
                for eb in range(EBLK):
                    nc.tensor.matmul(
                        acc[eb][:],
                        lhsT=inT_r[:, k, :],
                        rhs=pt[:, s, eb * NFREE : (eb + 1) * NFREE],
                        start=(k == 0),
                        stop=False,
                    )
        for ko in range(KTL):
            et = epool.tile([P, KSUB, EC], f32)
            nc.sync.dma_start(et[:], eT3[ko].rearrange("s p e -> p s e"))
            er = erpool.tile([P, KSUB, EC], f32r)
            nc.vector.tensor_copy(er[:], et[:])
            for s in range(KSUB):
                k = ko * KSUB + s
                for eb in range(EBLK):
                    nc.tensor.matmul(
                        acc[eb][:],
                        lhsT=lT_r[:, k, :],
                        rhs=er[:, s, eb * NFREE : (eb + 1) * NFREE],
                        start=False,
                        stop=(k == NV // P - 1),
                    )
        for eb in range(EBLK):
            ot = opool.tile([B, NFREE], f32)
            nc.scalar.mul(ot[:], acc[eb][:], 0.5)
            nc.sync.dma_start(out[:, eb * NFREE : (eb + 1) * NFREE], ot[:])

    nc.compile()
    return nc


def _get_nc():
    global _NC_CACHE
    if _NC_CACHE is None:
        _NC_CACHE = _build_nc()
    return _NC_CACHE


def _prepare_in_maps(input, input_weight, mask, llr, llr_weight, llr_expander):
    inp = np.ascontiguousarray(np.asarray(input, dtype=np.float32))
    mask = np.asarray(mask, dtype=np.float32)
    input_weight = np.asarray(input_weight, dtype=np.float32)
    llr_expander = np.asarray(llr_expander, dtype=np.float32)
    lw = np.asarray(llr_weight, dtype=np.float32) * np.asarray(llr, dtype=np.float32)

    inT = np.ascontiguousarray(inp.T)          # [E, B]
    lT = np.ascontiguousarray(lw.T)            # [NV, B]

    in_maps = []
    for c in range(NCORES):
        sl = slice(c * EC, (c + 1) * EC)
        in_maps.append(
            {
                "inT": inT,
                "lT": lT,
                "mT": np.ascontiguousarray(mask[sl].T),
                "wT": np.ascontiguousarray(input_weight[sl].T),
                "eT": np.ascontiguousarray(llr_expander[sl].T),
            }
        )
    return in_maps


def kernel(input, input_weight, mask, llr, llr_weight, llr_expander):
    from concourse.bass_utils import run_bass_kernel_spmd

    in_maps = _prepare_in_maps(
        input, input_weight, mask, llr, llr_weight, llr_expander
    )
    nc = _get_nc()
    res = run_bass_kernel_spmd(nc, in_maps, list(range(NCORES)))
    out = np.concatenate(
        [res.results[c]["out"] for c in range(NCORES)], axis=1
    )
    return np.ascontiguousarray(out, dtype=np.float32)
